# revision 9
# baseline (speedup 1.0000x reference)
"""DeepseekV3 attention on 8 TRN2 NeuronCores.

Sharding: phase 1 token-sharded latent projections (8 blocks of 512 tokens),
AllGather of latents within each 4-core batch group, phase 2 head-sharded
(4 heads per core) attention + partial o_proj; host sums the 4 partials
per batch. All matmuls bf16 with fp32 PSUM accumulation.

v4: kv latents (+ k rope) AllGathered first; batched single-DMA AG stores;
PE warmup matmuls during the initial weight DMA; attention on 256-token query
blocks with two heads interleaved per pipeline (hides ACT/DVE handoff
latency); sum-of-exp accumulated on the PE; o_proj evacuation on ScalarE.
"""
import numpy as np
import ml_dtypes

import concourse.bacc as bacc
import concourse.mybir as mybir
import concourse.tile as tile

B, T, HID = 2, 2048, 2048
NH = 16
QLR, KVLR = 1536, 512
DN, DR = 128, 64
DQK, DV = DN + DR, 128
EPS = 1e-6
THETA = 10000.0
SCALE = DQK ** -0.5

NB = 512          # tokens per phase-1 block
HPC = 4           # heads per core in phase 2
HB = NB // 2      # q-latent AllGather half (256 tokens)
KVF = 4 * NB + HB  # 2304 free elems per partition in kv AG buffer

f32 = mybir.dt.float32
bf16 = mybir.dt.bfloat16
Exp = mybir.ActivationFunctionType.Exp
Sqrt = mybir.ActivationFunctionType.Sqrt
Square = mybir.ActivationFunctionType.Square
Copy = mybir.ActivationFunctionType.Copy

_BF = ml_dtypes.bfloat16


def _build():
    nc = bacc.Bacc(None, num_devices=8)

    # ---- per-core inputs ----
    xT = nc.declare_dram_parameter("xT", [HID, NB], bf16, isOutput=False)
    wqa = nc.declare_dram_parameter("wqa", [HID, QLR], bf16, isOutput=False)
    wkva = nc.declare_dram_parameter("wkva", [HID, KVLR + 2 * DR], bf16, isOutput=False)
    wqb = nc.declare_dram_parameter("wqb", [QLR, 768], bf16, isOutput=False)
    sel = nc.declare_dram_parameter("sel", [2, 128, 128], bf16, isOutput=False)
    wkvk = nc.declare_dram_parameter("wkvk", [KVLR, 512], bf16, isOutput=False)
    wkvv = nc.declare_dram_parameter("wkvv", [KVLR, 512], bf16, isOutput=False)
    wo = nc.declare_dram_parameter("wo", [HPC * DV, HID], bf16, isOutput=False)
    cs = nc.declare_dram_parameter("cs", [128, T], bf16, isOutput=False)  # [c;c;-s;s]
    cso = nc.declare_dram_parameter("cso", [128, NB], bf16, isOutput=False)  # own block
    maskp = nc.declare_dram_parameter("maskp", [128, 128], bf16, isOutput=False)
    eye2 = nc.declare_dram_parameter("eye2", [128, 64], bf16, isOutput=False)
    out = nc.declare_dram_parameter("out", [T, HID], f32, isOutput=True)

    # AG buffers: [128 partitions, free] so phase-2 reads are 1 big DMA each.
    # kv: 4 latent m-tiles of 512 cols + roped krot packed [64,512]->[128,256]
    ag_in_kv = nc.dram_tensor("ag_in_kv", [128, KVF], bf16)
    ag_out_kv = nc.dram_tensor("ag_out_kv", [4, 128, KVF], bf16)
    ag_in_q0 = nc.dram_tensor("ag_in_q0", [128, 12, HB], bf16)
    ag_out_q0 = nc.dram_tensor("ag_out_q0", [4, 128, 12, HB], bf16)
    ag_in_q1 = nc.dram_tensor("ag_in_q1", [128, 12, HB], bf16)
    ag_out_q1 = nc.dram_tensor("ag_out_q1", [4, 128, 12, HB], bf16)

    with tile.TileContext(nc) as tc:
        # ============ phase 1: latents for own 512-token block ============
        with (
            tc.tile_pool(name="p1", bufs=1) as p1,
            tc.tile_pool(name="p1w", bufs=2) as p1w,
            tc.tile_pool(name="ps1", bufs=2, space="PSUM") as ps1,
            tc.tile_pool(name="ps1r", bufs=1, space="PSUM") as ps1r,
            tc.tile_pool(name="ps1acc", bufs=2, space="PSUM") as ps1acc,
        ):
            ones = p1.tile([128, 128], bf16, tag="ones")
            nc.vector.memset(ones[:], 1.0)
            # PE warmup: ~4us of junk matmuls while the input DMAs land, so
            # the HAM clock-gate is already at 8/8 when real matmuls start
            wps = ps1r.tile([128, 64], f32, tag="warm")
            for _ in range(48):
                nc.tensor.matmul(wps[:], ones[:], ones[:, 0:64],
                                 start=True, stop=True)

            xt = p1.tile([128, 16, NB], bf16, tag="xt")
            wkvat = p1.tile([128, 16, KVLR + 2 * DR], bf16, tag="wkvat")
            wqat = p1.tile([128, 16, QLR], bf16, tag="wqat")
            for k in range(16):
                nc.sync.dma_start(xt[:, k], xT[128 * k : 128 * (k + 1), :])
                nc.sync.dma_start(wkvat[:, k], wkva[128 * k : 128 * (k + 1), :])
            csot = p1.tile([128, NB], bf16, tag="csot")
            nc.sync.dma_start(csot[:], cso[:])
            eyet = p1.tile([128, 64], bf16, tag="eyet")
            nc.sync.dma_start(eyet[:], eye2[:])
            for k in range(16):
                nc.sync.dma_start(wqat[:, k], wqa[128 * k : 128 * (k + 1), :])

            def compute_inv(ssq, d):
                w = ssq.shape[1]
                mt_ = p1w.tile([128, NB], f32, tag="rmst", name="rmst")
                nc.vector.tensor_scalar(
                    mt_[:, :w], ssq[:], 1.0 / d, EPS,
                    mybir.AluOpType.mult, mybir.AluOpType.add,
                )
                rms = p1w.tile([128, NB], f32, tag="rms", name="rms")
                nc.scalar.activation(rms[:, :w], mt_[:, :w], Sqrt)
                inv = p1w.tile([128, NB], f32, tag="inv", name="inv")
                nc.vector.reciprocal(inv[:, :w], rms[:, :w])
                return inv

            # ---- ckv^T: m 0..3 kv_lat (normed), m 4 = rope(k_rot) ----
            kvlat = p1.tile([128, 4, NB], f32, tag="kvlat")
            ssq_kv = ps1acc.tile([128, NB], f32, tag="ssq")
            for m in range(5):
                ps = ps1.tile([128, NB], f32, tag="p1ps")
                for k in range(16):
                    nc.tensor.matmul(
                        ps[:], wkvat[:, k, 128 * m : 128 * (m + 1)], xt[:, k],
                        start=(k == 0), stop=(k == 15),
                    )
                if m < 4:
                    sq = p1w.tile([128, NB], bf16, tag="sq")
                    nc.scalar.activation(sq[:], ps[:], Square)
                    nc.vector.tensor_copy(kvlat[:, m], ps[:])
                    nc.tensor.matmul(ssq_kv[:], ones[:], sq[:],
                                     start=(m == 0), stop=(m == 3))
                else:
                    # rope k: ps rows = [kr; swap(kr)]; cso = [c;c;-s;s]
                    tt = p1w.tile([128, NB], bf16, tag="kropet")
                    nc.vector.tensor_mul(tt[:], ps[:], csot[:])
                    pr = ps1r.tile([64, NB], f32, tag="kropeps")
                    nc.tensor.matmul(pr[:], eyet[:], tt[:], start=True, stop=True)
                    rot = p1w.tile([64, NB], bf16, tag="rot")
                    nc.vector.tensor_copy(rot[:], pr[:])
                    nc.gpsimd.dma_start(
                        ag_in_kv[0:64, 4 * NB : 4 * NB + HB], rot[:, 0:HB])
                    nc.gpsimd.dma_start(
                        ag_in_kv[64:128, 4 * NB : 4 * NB + HB], rot[:, HB:NB])

            inv = compute_inv(ssq_kv, KVLR)
            nkv = p1w.tile([128, 4 * NB], bf16, tag="normkv")
            for m in range(4):
                nc.vector.tensor_mul(
                    nkv[:, NB * m : NB * (m + 1)], kvlat[:, m], inv[:, :NB])
            nc.gpsimd.dma_start(ag_in_kv[:, 0 : 4 * NB], nkv[:])
            nc.gpsimd.collective_compute(
                "AllGather", mybir.AluOpType.bypass,
                replica_groups=[[0, 1, 2, 3], [4, 5, 6, 7]],
                ins=[ag_in_kv[:]], outs=[ag_out_kv[:]],
            )

            # ---- q_lat^T in two column halves so AG_q0 starts early ----
            for half, (agi, ago_) in enumerate(
                ((ag_in_q0, ag_out_q0), (ag_in_q1, ag_out_q1))
            ):
                c0 = half * HB
                qlat = p1.tile([128, 12, HB], f32, tag="qlat", name="qlat")
                ssq_q = ps1acc.tile([128, HB], f32, tag="ssq", name="ssq_q")
                for m in range(12):
                    ps = ps1.tile([128, HB], f32, tag="p1psq", name="p1psq")
                    for k in range(16):
                        nc.tensor.matmul(
                            ps[:], wqat[:, k, 128 * m : 128 * (m + 1)],
                            xt[:, k, c0 : c0 + HB],
                            start=(k == 0), stop=(k == 15),
                        )
                    sq = p1w.tile([128, HB], bf16, tag="sqq", name="sqq")
                    nc.scalar.activation(sq[:], ps[:], Square)
                    nc.vector.tensor_copy(qlat[:, m], ps[:])
                    nc.tensor.matmul(ssq_q[:], ones[:], sq[:],
                                     start=(m == 0), stop=(m == 11))
                inv = compute_inv(ssq_q, QLR)
                nq = p1w.tile([128, 12, HB], bf16, tag="normq", name="normq")
                for m in range(12):
                    nc.vector.tensor_mul(nq[:, m], qlat[:, m], inv[:, :HB])
                nc.gpsimd.dma_start(agi[:], nq[:])
                nc.gpsimd.collective_compute(
                    "AllGather", mybir.AluOpType.bypass,
                    replica_groups=[[0, 1, 2, 3], [4, 5, 6, 7]],
                    ins=[agi[:]], outs=[ago_[:]],
                )

        # ============ phase 2: 4 heads, full batch ============
        with tc.tile_pool(name="p2", bufs=1) as p2:
            qTp = p2.tile([128, 4, 4, NB], bf16, tag="qTp")      # pass, per head m
            qrot = [
                p2.tile([128, T], bf16, tag=f"qrot{h}", name=f"qrot{h}")
                for h in range(HPC)
            ]
            krotT2 = p2.tile([128, T], bf16, tag="krotT2")       # [kr;kr] stacked
            kpT = p2.tile([128, 4, 4, NB], bf16, tag="kpT")      # [d, head, r, t]
            vT = p2.tile([128, 16, 512], bf16, tag="vT")         # [t, t-tile, dv]
            attnT = p2.tile([128, 4, 8, HB], bf16, tag="attnT")  # [dv, head, qb, t]
            cst = p2.tile([128, T], bf16, tag="cst")
            nc.sync.dma_start(cst[:], cs[:])
            maskt = p2.tile([128, 128], bf16, tag="maskt")
            nc.sync.dma_start(maskt[:], maskp[:])
            onesb = p2.tile([128, 128], bf16, tag="onesb")
            nc.vector.memset(onesb[:], 1.0)
            wot = p2.tile([128, 4, HID], bf16, tag="wot")
            for k in range(4):
                nc.sync.dma_start(wot[:, k], wo[128 * k : 128 * (k + 1), :])

            # ---- 2b: k_pass^T, V (needs AG_kv) ----
            with (
                tc.tile_pool(name="p2b", bufs=1) as p2b,
                tc.tile_pool(name="p2bl", bufs=2) as p2bl,
                tc.tile_pool(name="ps2b", bufs=2, space="PSUM") as ps2b,
            ):
                wkkt = p2b.tile([128, 4, 512], bf16, tag="wkkt")
                wkvt = p2b.tile([128, 4, 512], bf16, tag="wkvt")
                for k in range(4):
                    nc.sync.dma_start(wkkt[:, k], wkvk[128 * k : 128 * (k + 1), :])
                    nc.sync.dma_start(wkvt[:, k], wkvv[128 * k : 128 * (k + 1), :])
                for r in range(4):
                    kvl = p2bl.tile([128, 4, NB], bf16, tag="kvl", name="kvl")
                    nc.sync.dma_start(kvl[:], ag_out_kv[r, :, 0 : 4 * NB])
                    for dup in (0, 1):
                        nc.sync.dma_start(
                            krotT2[64 * dup : 64 * dup + 64,
                                   NB * r : NB * r + HB],
                            ag_out_kv[r, 0:64, 4 * NB : 4 * NB + HB])
                        nc.sync.dma_start(
                            krotT2[64 * dup : 64 * dup + 64,
                                   NB * r + HB : NB * (r + 1)],
                            ag_out_kv[r, 64:128, 4 * NB : 4 * NB + HB])
                    # k_pass^T for 4 heads
                    for m in range(4):
                        ps = ps2b.tile([128, NB], f32, tag="k2ps")
                        for k in range(4):
                            nc.tensor.matmul(
                                ps[:], wkkt[:, k, 128 * m : 128 * (m + 1)],
                                kvl[:, k], start=(k == 0), stop=(k == 3),
                            )
                        nc.vector.tensor_copy(kpT[:, m, r], ps[:])
                    # V token-major
                    for s in range(4):
                        ps = ps2b.tile([128, 512], f32, tag="v2ps")
                        for k in range(4):
                            nc.tensor.matmul(
                                ps[:], kvl[:, k, 128 * s : 128 * (s + 1)],
                                wkvt[:, k], start=(k == 0), stop=(k == 3),
                            )
                        nc.vector.tensor_copy(vT[:, 4 * r + s], ps[:])

            # ---- 2a + attention + o_proj, pipelined per (half, r) ----
            # PSUM: psS 4 (q2ps/scps/selps/ops shared) + psSum 2 + psB 2 = 8
            with (
                tc.tile_pool(name="p2a", bufs=1) as p2a,
                tc.tile_pool(name="p2as", bufs=2) as p2as,
                tc.tile_pool(name="p2d", bufs=5) as p2d,
                tc.tile_pool(name="p2dn", bufs=2) as p2dn,
                tc.tile_pool(name="p2eo", bufs=3) as p2eo,
                tc.tile_pool(name="psS", bufs=4, space="PSUM") as psS,
                tc.tile_pool(name="psSum", bufs=2, space="PSUM") as psSum,
                tc.tile_pool(name="psB", bufs=2, space="PSUM") as psB,
            ):
                wqbt = p2a.tile([128, 12, 768], bf16, tag="wqbt")
                selt = p2a.tile([128, 2, 128], bf16, tag="selt")
                for v in range(2):
                    nc.sync.dma_start(selt[:, v], sel[v])
                for k in range(12):
                    nc.sync.dma_start(wqbt[:, k], wqb[128 * k : 128 * (k + 1), :])
                qrw = p2a.tile([128, 2, 4, NB], bf16, tag="qrw")  # raw rot pairs

                def attention_pair(h2, qb):
                    nkt = 2 * qb + 2
                    aps, sps = {}, {}
                    for h in h2:
                        aps[h] = psB.tile([128, HB], f32, tag="attn_ps",
                                          name=f"aps{h}")
                        sps[h] = psSum.tile([128, HB], f32, tag="sums",
                                            name=f"sps{h}")
                    pend = []

                    def emit_pv(h, pe, pk, off):
                        nc.tensor.matmul(
                            aps[h][:, off:HB], vT[:, pk, 128 * h : 128 * (h + 1)],
                            pe[:, off:HB], start=(pk == 0), stop=(pk == nkt - 1),
                        )

                    for kt in range(nkt):
                        m = kt - 2 * qb
                        off = 128 * m if m > 0 else 0
                        rk, sl = kt // 4, 128 * (kt % 4)
                        for h in h2:
                            scp = psS.tile([128, HB], f32, tag="big",
                                           name="scps")
                            nc.tensor.matmul(
                                scp[:, off:HB], kpT[:, h, rk, sl : sl + 128],
                                qTp[:, h, qb // 2,
                                    (qb % 2) * HB + off : (qb % 2 + 1) * HB],
                                start=True, stop=False,
                            )
                            nc.tensor.matmul(
                                scp[:, off:HB],
                                krotT2[:, 128 * kt : 128 * kt + 128],
                                qrot[h][:, HB * qb + off : HB * (qb + 1)],
                                start=False, stop=True,
                            )
                            et = p2d.tile([128, HB], bf16, tag="expT")
                            nc.scalar.activation(
                                et[:, off:HB], scp[:, off:HB], Exp, scale=SCALE
                            )
                            if m >= 0:
                                nc.vector.tensor_mul(
                                    et[:, off : off + 128],
                                    et[:, off : off + 128], maskt[:],
                                )
                            nc.tensor.matmul(
                                sps[h][:, off:HB], onesb[:], et[:, off:HB],
                                start=(kt == 0), stop=(kt == nkt - 1),
                            )
                            if len(pend) == 2:
                                emit_pv(*pend.pop(0))
                            pend.append((h, et, kt, off))
                    for p_ in pend:
                        emit_pv(*p_)
                    for h in h2:
                        rec = p2dn.tile([128, HB], f32, tag="rec",
                                        name=f"rec{h}")
                        nc.vector.reciprocal(rec[:], sps[h][:])
                        nc.vector.tensor_mul(attnT[:, h, qb], aps[h][:], rec[:])

                for c, agq in ((0, ag_out_q0), (1, ag_out_q1)):
                    for r in range(4):
                        qlt = p2as.tile([128, 12, HB], bf16, tag="qlt")
                        nc.sync.dma_start(qlt[:], agq[r])
                        for m in range(6):
                            ps = psS.tile([128, HB], f32, tag="big", name="q2ps")
                            for k in range(12):
                                nc.tensor.matmul(
                                    ps[:], wqbt[:, k, 128 * m : 128 * (m + 1)],
                                    qlt[:, k], start=(k == 0), stop=(k == 11),
                                )
                            dst = qTp[:, m, r] if m < 4 else qrw[:, m - 4, r]
                            nc.any.tensor_copy(
                                dst[:, c * HB : (c + 1) * HB], ps[:]
                            )
                        # rope q: sel-matmul + CS-mul; the [rot;swap] pair-
                        # combine folds into the score matmul via krotT2
                        for h in range(HPC):
                            sp = psS.tile([128, HB], f32, tag="big",
                                          name="selps")
                            nc.tensor.matmul(
                                sp[:], selt[:, h % 2],
                                qrw[:, h // 2, r, c * HB : (c + 1) * HB],
                                start=True, stop=True)
                            nc.vector.tensor_mul(
                                qrot[h][:, NB * r + c * HB : NB * r + (c + 1) * HB],
                                sp[:],
                                cst[:, NB * r + c * HB : NB * r + (c + 1) * HB],
                            )
                        qb = 2 * r + c
                        attention_pair((0, 1), qb)
                        attention_pair((2, 3), qb)
                        # o_proj partial for this query block
                        for half2 in range(2):
                            tm = 4 * r + 2 * c + half2
                            s = 128 * half2
                            for n in range(4):
                                ps = psS.tile([128, 512], f32, tag="big",
                                              name="ops")
                                for k in range(4):
                                    nc.tensor.matmul(
                                        ps[:], attnT[:, k, qb, s : s + 128],
                                        wot[:, k, 512 * n : 512 * (n + 1)],
                                        start=(k == 0), stop=(k == 3),
                                    )
                                ot = p2eo.tile([128, 512], f32, tag="oT")
                                nc.scalar.activation(ot[:], ps[:], Copy)
                                nc.gpsimd.dma_start(
                                    out[128 * tm : 128 * (tm + 1),
                                        512 * n : 512 * (n + 1)], ot[:],
                                )

    nc.finalize()
    return nc


_NC = None


def _get_nc():
    global _NC
    if _NC is None:
        _NC = _build()
    return _NC


def _prep_inputs(x, attention_mask, positions, wqa, qa_scale, wqb, wkva,
                 kva_scale, wkvb, wo):
    x = np.asarray(x, np.float32)
    positions = np.asarray(positions)
    wqa = np.asarray(wqa, np.float32)
    wqb = np.asarray(wqb, np.float32) * np.asarray(qa_scale, np.float32)[:, None]
    wkva = np.asarray(wkva, np.float32)
    wkvb = np.asarray(wkvb, np.float32) * np.asarray(kva_scale, np.float32)[:, None]
    wo = np.asarray(wo, np.float32)

    # wkva augmented with swapped-rot columns
    kr = wkva[:, KVLR:]
    wkva_aug = np.concatenate(
        [wkva[:, :KVLR], kr, kr[:, DR // 2 :], kr[:, : DR // 2]], axis=1
    ).astype(_BF)

    # triu mask for the 128-col diagonal window: mask[r, c] = c >= r
    rr = np.arange(128)[:, None]
    cc = np.arange(128)[None, :]
    maskp = (cc >= rr).astype(_BF)

    eye2 = np.concatenate([np.eye(64), np.eye(64)], axis=0).astype(_BF)

    # sel[v]: out rows [0:64]=src rows [64v:64v+64]; [64:128]=32-swapped copy
    sel = np.zeros((2, 128, 128), np.float32)
    for v in range(2):
        for i in range(64):
            sel[v, 64 * v + i, i] = 1.0
            sel[v, 64 * v + ((i + 32) % 64), 64 + i] = 1.0
    sel = sel.astype(_BF)

    # per-batch cos/sin stack [c; c; -s; s]
    inv_freq = 1.0 / (THETA ** (np.arange(0, DR, 2, dtype=np.float32) / DR))
    cs_b = []
    for b in range(B):
        ang = positions[b].astype(np.float32)[None, :] * inv_freq[:, None]
        c, s = np.cos(ang), np.sin(ang)
        cs_b.append(np.concatenate([c, c, -s, s], axis=0).astype(_BF))

    wqa_bf = wqa.astype(_BF)
    in_maps = []
    for core in range(8):
        b, j = core // 4, core % 4
        hs = [4 * (core % 4) + i for i in range(HPC)]
        # wqb per head-group: [pass x4 | rot per head x4]
        cols = [wqb[:, h * DQK : h * DQK + DN] for h in hs]
        for h in hs:
            cols.append(wqb[:, h * DQK + DN : (h + 1) * DQK])
        wqb_hg = np.concatenate(cols, axis=1).astype(_BF)
        wkvk_hg = np.concatenate(
            [wkvb[:, h * (DN + DV) : h * (DN + DV) + DN] for h in hs], axis=1
        ).astype(_BF)
        wkvv_hg = np.concatenate(
            [wkvb[:, h * (DN + DV) + DN : (h + 1) * (DN + DV)] for h in hs], axis=1
        ).astype(_BF)
        wo_hg = wo[hs[0] * DV : (hs[-1] + 1) * DV, :].astype(_BF)
        xTb = np.ascontiguousarray(
            x[b, NB * j : NB * (j + 1), :].T).astype(_BF)
        in_maps.append({
            "xT": xTb,
            "wqa": wqa_bf,
            "wkva": wkva_aug,
            "wqb": wqb_hg,
            "wkvk": wkvk_hg,
            "wkvv": wkvv_hg,
            "wo": wo_hg,
            "cs": cs_b[b],
            "cso": np.ascontiguousarray(cs_b[b][:, NB * j : NB * (j + 1)]),
            "maskp": maskp,
            "eye2": eye2,
            "sel": sel,
        })
    return in_maps


def _run(inputs, trace=False, trace_kwargs=None):
    from concourse.bass_utils import run_bass_kernel_spmd

    nc = _get_nc()
    in_maps = _prep_inputs(**inputs)
    res = run_bass_kernel_spmd(
        nc, in_maps, list(range(8)), trace=trace,
        trace_kwargs=trace_kwargs or {},
    )
    outs = np.zeros((B, T, HID), np.float32)
    for core in range(8):
        outs[core // 4] += res.results[core]["out"]
    return outs, res


def kernel(**inputs) -> np.ndarray:
    out, _ = _run(inputs)
    return out


# revision 14
# speedup vs baseline: 1.0403x; 1.0403x over previous
"""DeepseekV3 attention on 8 TRN2 NeuronCores.

Sharding: phase 1 token-sharded latent projections (8 blocks of 512 tokens),
AllGather of latents within each 4-core batch group, phase 2 head-sharded
(4 heads per core) attention + partial o_proj; host sums the 4 partials
per batch. All matmuls bf16 with fp32 PSUM accumulation.

v4: kv latents (+ k rope) AllGathered first; batched single-DMA AG stores;
PE warmup matmuls during the initial weight DMA; attention on 256-token query
blocks with two heads interleaved per pipeline (hides ACT/DVE handoff
latency); sum-of-exp accumulated on the PE; o_proj evacuation on ScalarE.
"""
import numpy as np
import ml_dtypes

import concourse.bacc as bacc
import concourse.mybir as mybir
import concourse.tile as tile

B, T, HID = 2, 2048, 2048
NH = 16
QLR, KVLR = 1536, 512
DN, DR = 128, 64
DQK, DV = DN + DR, 128
EPS = 1e-6
THETA = 10000.0
SCALE = DQK ** -0.5

NB = 512          # tokens per phase-1 block
HPC = 4           # heads per core in phase 2
HB = NB // 2      # q-latent AllGather half (256 tokens)
KVF = 4 * NB + HB  # 2304 free elems per partition in kv AG buffer

f32 = mybir.dt.float32
bf16 = mybir.dt.bfloat16
Exp = mybir.ActivationFunctionType.Exp
Sqrt = mybir.ActivationFunctionType.Sqrt
Square = mybir.ActivationFunctionType.Square
Copy = mybir.ActivationFunctionType.Copy

_BF = ml_dtypes.bfloat16


def _build():
    nc = bacc.Bacc(None, num_devices=8)

    # ---- per-core inputs ----
    xT = nc.declare_dram_parameter("xT", [HID, NB], bf16, isOutput=False)
    wqa = nc.declare_dram_parameter("wqa", [HID, QLR], bf16, isOutput=False)
    wkva = nc.declare_dram_parameter("wkva", [HID, KVLR + 2 * DR], bf16, isOutput=False)
    wqb = nc.declare_dram_parameter("wqb", [QLR, 768], bf16, isOutput=False)
    sel = nc.declare_dram_parameter("sel", [2, 128, 128], bf16, isOutput=False)
    wkvk = nc.declare_dram_parameter("wkvk", [KVLR, 512], bf16, isOutput=False)
    wkvv = nc.declare_dram_parameter("wkvv", [KVLR, 512], bf16, isOutput=False)
    wo = nc.declare_dram_parameter("wo", [HPC * DV, HID], bf16, isOutput=False)
    cs = nc.declare_dram_parameter("cs", [128, T], bf16, isOutput=False)  # [c;c;-s;s]
    cso = nc.declare_dram_parameter("cso", [128, NB], bf16, isOutput=False)  # own block
    maskp = nc.declare_dram_parameter("maskp", [128, 128], bf16, isOutput=False)
    eye2 = nc.declare_dram_parameter("eye2", [128, 64], bf16, isOutput=False)
    out = nc.declare_dram_parameter("out", [T, HID], f32, isOutput=True)

    # AG buffers: [128 partitions, free] so phase-2 reads are 1 big DMA each.
    # kv: 4 latent m-tiles of 512 cols + roped krot packed [64,512]->[128,256]
    ag_in_kv = nc.dram_tensor("ag_in_kv", [128, KVF], bf16)
    ag_out_kv = nc.dram_tensor("ag_out_kv", [4, 128, KVF], bf16)
    ag_in_q0 = nc.dram_tensor("ag_in_q0", [128, 12, HB], bf16)
    ag_out_q0 = nc.dram_tensor("ag_out_q0", [4, 128, 12, HB], bf16)
    ag_in_q1 = nc.dram_tensor("ag_in_q1", [128, 12, HB], bf16)
    ag_out_q1 = nc.dram_tensor("ag_out_q1", [4, 128, 12, HB], bf16)

    with tile.TileContext(nc) as tc:
        # ============ phase 1: latents for own 512-token block ============
        with (
            tc.tile_pool(name="p1", bufs=1) as p1,
            tc.tile_pool(name="p1w", bufs=2) as p1w,
            tc.tile_pool(name="ps1", bufs=2, space="PSUM") as ps1,
            tc.tile_pool(name="ps1r", bufs=1, space="PSUM") as ps1r,
            tc.tile_pool(name="ps1acc", bufs=2, space="PSUM") as ps1acc,
        ):
            ones = p1.tile([128, 128], bf16, tag="ones")
            nc.vector.memset(ones[:], 1.0)
            # PE warmup: ~4us of junk matmuls while the input DMAs land, so
            # the HAM clock-gate is already at 8/8 when real matmuls start
            wps = ps1r.tile([128, 64], f32, tag="warm")
            for _ in range(48):
                nc.tensor.matmul(wps[:], ones[:], ones[:, 0:64],
                                 start=True, stop=True)

            xt = p1.tile([128, 16, NB], bf16, tag="xt")
            wkvat = p1.tile([128, 16, KVLR + 2 * DR], bf16, tag="wkvat")
            wqat = p1.tile([128, 16, QLR], bf16, tag="wqat")
            for k in range(16):
                nc.sync.dma_start(xt[:, k], xT[128 * k : 128 * (k + 1), :])
                nc.sync.dma_start(wkvat[:, k], wkva[128 * k : 128 * (k + 1), :])
            csot = p1.tile([128, NB], bf16, tag="csot")
            nc.sync.dma_start(csot[:], cso[:])
            eyet = p1.tile([128, 64], bf16, tag="eyet")
            nc.sync.dma_start(eyet[:], eye2[:])
            for k in range(16):
                nc.sync.dma_start(wqat[:, k], wqa[128 * k : 128 * (k + 1), :])

            def compute_inv(ssq, d):
                w = ssq.shape[1]
                mt_ = p1w.tile([128, NB], f32, tag="rmst", name="rmst")
                nc.vector.tensor_scalar(
                    mt_[:, :w], ssq[:], 1.0 / d, EPS,
                    mybir.AluOpType.mult, mybir.AluOpType.add,
                )
                rms = p1w.tile([128, NB], f32, tag="rms", name="rms")
                nc.scalar.activation(rms[:, :w], mt_[:, :w], Sqrt)
                inv = p1w.tile([128, NB], f32, tag="inv", name="inv")
                nc.vector.reciprocal(inv[:, :w], rms[:, :w])
                return inv

            # ---- ckv^T: m 0..3 kv_lat (normed), m 4 = rope(k_rot) ----
            kvlat = p1.tile([128, 4, NB], f32, tag="kvlat")
            ssq_kv = ps1acc.tile([128, NB], f32, tag="ssq")
            for m in range(5):
                ps = ps1.tile([128, NB], f32, tag="p1ps")
                for k in range(16):
                    nc.tensor.matmul(
                        ps[:], wkvat[:, k, 128 * m : 128 * (m + 1)], xt[:, k],
                        start=(k == 0), stop=(k == 15),
                    )
                if m < 4:
                    sq = p1w.tile([128, NB], bf16, tag="sq")
                    nc.scalar.activation(sq[:], ps[:], Square)
                    nc.vector.tensor_copy(kvlat[:, m], ps[:])
                    nc.tensor.matmul(ssq_kv[:], ones[:], sq[:],
                                     start=(m == 0), stop=(m == 3))
                else:
                    # rope k: ps rows = [kr; swap(kr)]; cso = [c;c;-s;s]
                    tt = p1w.tile([128, NB], bf16, tag="kropet")
                    nc.vector.tensor_mul(tt[:], ps[:], csot[:])
                    pr = ps1r.tile([64, NB], f32, tag="kropeps")
                    nc.tensor.matmul(pr[:], eyet[:], tt[:], start=True, stop=True)
                    rot = p1w.tile([64, NB], bf16, tag="rot")
                    nc.vector.tensor_copy(rot[:], pr[:])
                    nc.gpsimd.dma_start(
                        ag_in_kv[0:64, 4 * NB : 4 * NB + HB], rot[:, 0:HB])
                    nc.gpsimd.dma_start(
                        ag_in_kv[64:128, 4 * NB : 4 * NB + HB], rot[:, HB:NB])

            inv = compute_inv(ssq_kv, KVLR)
            nkv = p1w.tile([128, 4 * NB], bf16, tag="normkv")
            for m in range(4):
                nc.vector.tensor_mul(
                    nkv[:, NB * m : NB * (m + 1)], kvlat[:, m], inv[:, :NB])
            nc.gpsimd.dma_start(ag_in_kv[:, 0 : 4 * NB], nkv[:])
            nc.gpsimd.collective_compute(
                "AllGather", mybir.AluOpType.bypass,
                replica_groups=[[0, 1, 2, 3], [4, 5, 6, 7]],
                ins=[ag_in_kv[:]], outs=[ag_out_kv[:]],
            )

            # ---- q_lat^T in two column halves so AG_q0 starts early ----
            for half, (agi, ago_) in enumerate(
                ((ag_in_q0, ag_out_q0), (ag_in_q1, ag_out_q1))
            ):
                c0 = half * HB
                qlat = p1.tile([128, 12, HB], f32, tag="qlat", name="qlat")
                ssq_q = ps1acc.tile([128, HB], f32, tag="ssq", name="ssq_q")
                for m in range(12):
                    ps = ps1.tile([128, HB], f32, tag="p1psq", name="p1psq")
                    for k in range(16):
                        nc.tensor.matmul(
                            ps[:], wqat[:, k, 128 * m : 128 * (m + 1)],
                            xt[:, k, c0 : c0 + HB],
                            start=(k == 0), stop=(k == 15),
                        )
                    nc.vector.tensor_copy(qlat[:, m], ps[:])
                    sq = p1w.tile([128, HB], bf16, tag="sqq", name="sqq")
                    nc.vector.tensor_mul(sq[:], qlat[:, m], qlat[:, m])
                    nc.tensor.matmul(ssq_q[:], ones[:], sq[:],
                                     start=(m == 0), stop=(m == 11))
                inv = compute_inv(ssq_q, QLR)
                nq = p1w.tile([128, 12, HB], bf16, tag="normq", name="normq")
                for m in range(12):
                    nc.vector.tensor_mul(nq[:, m], qlat[:, m], inv[:, :HB])
                nc.gpsimd.dma_start(agi[:], nq[:])
                nc.gpsimd.collective_compute(
                    "AllGather", mybir.AluOpType.bypass,
                    replica_groups=[[0, 1, 2, 3], [4, 5, 6, 7]],
                    ins=[agi[:]], outs=[ago_[:]],
                )

        # ============ phase 2: 4 heads, full batch ============
        with tc.tile_pool(name="p2", bufs=1) as p2:
            qTp = p2.tile([128, 4, 4, NB], bf16, tag="qTp")      # pass, per head m
            qrot = [
                p2.tile([128, T], bf16, tag=f"qrot{h}", name=f"qrot{h}")
                for h in range(HPC)
            ]
            krotT2 = p2.tile([128, T], bf16, tag="krotT2")       # [kr;kr] stacked
            kpT = p2.tile([128, 4, 4, NB], bf16, tag="kpT")      # [d, head, r, t]
            vT = p2.tile([128, 16, 512], bf16, tag="vT")         # [t, t-tile, dv]
            attnT = p2.tile([128, 4, 8, HB], bf16, tag="attnT")  # [dv, head, qb, t]
            cst = p2.tile([128, T], bf16, tag="cst")
            nc.sync.dma_start(cst[:], cs[:])
            maskt = p2.tile([128, 128], bf16, tag="maskt")
            nc.sync.dma_start(maskt[:], maskp[:])
            onesb = p2.tile([128, 128], bf16, tag="onesb")
            nc.vector.memset(onesb[:], 1.0)
            wot = p2.tile([128, 4, HID], bf16, tag="wot")
            for k in range(4):
                nc.sync.dma_start(wot[:, k], wo[128 * k : 128 * (k + 1), :])

            # ---- 2b: k_pass^T, V (needs AG_kv) ----
            with (
                tc.tile_pool(name="p2b", bufs=1) as p2b,
                tc.tile_pool(name="p2bl", bufs=2) as p2bl,
                tc.tile_pool(name="ps2b", bufs=2, space="PSUM") as ps2b,
            ):
                wkkt = p2b.tile([128, 4, 512], bf16, tag="wkkt")
                wkvt = p2b.tile([128, 4, 512], bf16, tag="wkvt")
                for k in range(4):
                    nc.sync.dma_start(wkkt[:, k], wkvk[128 * k : 128 * (k + 1), :])
                    nc.sync.dma_start(wkvt[:, k], wkvv[128 * k : 128 * (k + 1), :])
                for r in range(4):
                    kvl = p2bl.tile([128, 4, NB], bf16, tag="kvl", name="kvl")
                    nc.sync.dma_start(kvl[:], ag_out_kv[r, :, 0 : 4 * NB])
                    for dup in (0, 1):
                        nc.sync.dma_start(
                            krotT2[64 * dup : 64 * dup + 64,
                                   NB * r : NB * r + HB],
                            ag_out_kv[r, 0:64, 4 * NB : 4 * NB + HB])
                        nc.sync.dma_start(
                            krotT2[64 * dup : 64 * dup + 64,
                                   NB * r + HB : NB * (r + 1)],
                            ag_out_kv[r, 64:128, 4 * NB : 4 * NB + HB])
                    # k_pass^T for 4 heads
                    for m in range(4):
                        ps = ps2b.tile([128, NB], f32, tag="k2ps")
                        for k in range(4):
                            nc.tensor.matmul(
                                ps[:], wkkt[:, k, 128 * m : 128 * (m + 1)],
                                kvl[:, k], start=(k == 0), stop=(k == 3),
                            )
                        nc.vector.tensor_copy(kpT[:, m, r], ps[:])
                    # V token-major
                    for s in range(4):
                        ps = ps2b.tile([128, 512], f32, tag="v2ps")
                        for k in range(4):
                            nc.tensor.matmul(
                                ps[:], kvl[:, k, 128 * s : 128 * (s + 1)],
                                wkvt[:, k], start=(k == 0), stop=(k == 3),
                            )
                        nc.vector.tensor_copy(vT[:, 4 * r + s], ps[:])

            # ---- 2a + attention + o_proj, pipelined per (half, r) ----
            # PSUM: psS 4 (q2ps/scps/selps/ops shared) + psSum 2 + psB 2 = 8
            with (
                tc.tile_pool(name="p2a", bufs=1) as p2a,
                tc.tile_pool(name="p2as", bufs=2) as p2as,
                tc.tile_pool(name="p2d", bufs=5) as p2d,
                tc.tile_pool(name="p2dn", bufs=3) as p2dn,
                tc.tile_pool(name="p2eo", bufs=3) as p2eo,
                tc.tile_pool(name="psS", bufs=6, space="PSUM") as psS,
                tc.tile_pool(name="psB", bufs=2, space="PSUM") as psB,
            ):
                wqbt = p2a.tile([128, 12, 768], bf16, tag="wqbt")
                selt = p2a.tile([128, 2, 128], bf16, tag="selt")
                for v in range(2):
                    nc.sync.dma_start(selt[:, v], sel[v])
                for k in range(12):
                    nc.sync.dma_start(wqbt[:, k], wqb[128 * k : 128 * (k + 1), :])
                qrw = p2a.tile([128, 2, 4, NB], bf16, tag="qrw")  # raw rot pairs

                def attention_pair(h2, qb):
                    nkt = 2 * qb + 2
                    aps, eac = {}, {}
                    for h in h2:
                        aps[h] = psB.tile([128, HB], f32, tag="attn_ps",
                                          name=f"aps{h}")
                        eac[h] = p2dn.tile([128, HB], bf16, tag="eacc",
                                           name=f"eac{h}")
                    pend = []

                    def emit_pv(h, pe, pk, off):
                        nc.tensor.matmul(
                            aps[h][:, off:HB], vT[:, pk, 128 * h : 128 * (h + 1)],
                            pe[:, off:HB], start=(pk == 0), stop=(pk == nkt - 1),
                        )

                    for kt in range(nkt):
                        m = kt - 2 * qb
                        off = 128 * m if m > 0 else 0
                        rk, sl = kt // 4, 128 * (kt % 4)
                        for h in h2:
                            scp = psS.tile([128, HB], f32, tag="big",
                                           name="scps")
                            nc.tensor.matmul(
                                scp[:, off:HB], kpT[:, h, rk, sl : sl + 128],
                                qTp[:, h, qb // 2,
                                    (qb % 2) * HB + off : (qb % 2 + 1) * HB],
                                start=True, stop=False,
                            )
                            nc.tensor.matmul(
                                scp[:, off:HB],
                                krotT2[:, 128 * kt : 128 * kt + 128],
                                qrot[h][:, HB * qb + off : HB * (qb + 1)],
                                start=False, stop=True,
                            )
                            et = p2d.tile([128, HB], bf16, tag="expT")
                            nc.scalar.activation(
                                et[:, off:HB], scp[:, off:HB], Exp, scale=SCALE
                            )
                            if m >= 0:
                                nc.vector.tensor_mul(
                                    et[:, off : off + 128],
                                    et[:, off : off + 128], maskt[:],
                                )
                            if kt == 0:
                                nc.vector.tensor_copy(eac[h][:], et[:])
                            else:
                                nc.vector.tensor_add(
                                    eac[h][:, off:HB], eac[h][:, off:HB],
                                    et[:, off:HB],
                                )
                            if len(pend) == 2:
                                emit_pv(*pend.pop(0))
                            pend.append((h, et, kt, off))
                    for p_ in pend:
                        emit_pv(*p_)
                    for h in h2:
                        sps = psS.tile([128, HB], f32, tag="big",
                                       name=f"sps{h}")
                        nc.tensor.matmul(sps[:], onesb[:], eac[h][:],
                                         start=True, stop=True)
                        rec = p2dn.tile([128, HB], f32, tag="rec",
                                        name=f"rec{h}")
                        nc.vector.reciprocal(rec[:], sps[:])
                        nc.vector.tensor_mul(attnT[:, h, qb], aps[h][:], rec[:])

                for c, agq in ((0, ag_out_q0), (1, ag_out_q1)):
                    for r in range(4):
                        # scalar HWDGE ring: keeps these AG_q-gated loads out
                        # of the sync-engine DMA FIFO (head-of-line blocking)
                        qlt = p2as.tile([128, 12, HB], bf16, tag="qlt")
                        nc.scalar.dma_start(qlt[:], agq[r])
                        for m in range(6):
                            ps = psS.tile([128, HB], f32, tag="big", name="q2ps")
                            for k in range(12):
                                nc.tensor.matmul(
                                    ps[:], wqbt[:, k, 128 * m : 128 * (m + 1)],
                                    qlt[:, k], start=(k == 0), stop=(k == 11),
                                )
                            dst = qTp[:, m, r] if m < 4 else qrw[:, m - 4, r]
                            nc.any.tensor_copy(
                                dst[:, c * HB : (c + 1) * HB], ps[:]
                            )
                        # rope q: sel-matmul + CS-mul; the [rot;swap] pair-
                        # combine folds into the score matmul via krotT2
                        for h in range(HPC):
                            sp = psS.tile([128, HB], f32, tag="big",
                                          name="selps")
                            nc.tensor.matmul(
                                sp[:], selt[:, h % 2],
                                qrw[:, h // 2, r, c * HB : (c + 1) * HB],
                                start=True, stop=True)
                            nc.vector.tensor_mul(
                                qrot[h][:, NB * r + c * HB : NB * r + (c + 1) * HB],
                                sp[:],
                                cst[:, NB * r + c * HB : NB * r + (c + 1) * HB],
                            )
                        qb = 2 * r + c
                        attention_pair((0, 1), qb)
                        attention_pair((2, 3), qb)
                        # o_proj partial for this query block
                        for half2 in range(2):
                            tm = 4 * r + 2 * c + half2
                            s = 128 * half2
                            for n in range(4):
                                ps = psS.tile([128, 512], f32, tag="big",
                                              name="ops")
                                for k in range(4):
                                    nc.tensor.matmul(
                                        ps[:], attnT[:, k, qb, s : s + 128],
                                        wot[:, k, 512 * n : 512 * (n + 1)],
                                        start=(k == 0), stop=(k == 3),
                                    )
                                ot = p2eo.tile([128, 512], f32, tag="oT")
                                nc.scalar.activation(ot[:], ps[:], Copy)
                                nc.gpsimd.dma_start(
                                    out[128 * tm : 128 * (tm + 1),
                                        512 * n : 512 * (n + 1)], ot[:],
                                )

    nc.finalize()
    return nc


_NC = None


def _get_nc():
    global _NC
    if _NC is None:
        _NC = _build()
    return _NC


def _prep_inputs(x, attention_mask, positions, wqa, qa_scale, wqb, wkva,
                 kva_scale, wkvb, wo):
    x = np.asarray(x, np.float32)
    positions = np.asarray(positions)
    wqa = np.asarray(wqa, np.float32)
    wqb = np.asarray(wqb, np.float32) * np.asarray(qa_scale, np.float32)[:, None]
    wkva = np.asarray(wkva, np.float32)
    wkvb = np.asarray(wkvb, np.float32) * np.asarray(kva_scale, np.float32)[:, None]
    wo = np.asarray(wo, np.float32)

    # wkva augmented with swapped-rot columns
    kr = wkva[:, KVLR:]
    wkva_aug = np.concatenate(
        [wkva[:, :KVLR], kr, kr[:, DR // 2 :], kr[:, : DR // 2]], axis=1
    ).astype(_BF)

    # triu mask for the 128-col diagonal window: mask[r, c] = c >= r
    rr = np.arange(128)[:, None]
    cc = np.arange(128)[None, :]
    maskp = (cc >= rr).astype(_BF)

    eye2 = np.concatenate([np.eye(64), np.eye(64)], axis=0).astype(_BF)

    # sel[v]: out rows [0:64]=src rows [64v:64v+64]; [64:128]=32-swapped copy
    sel = np.zeros((2, 128, 128), np.float32)
    for v in range(2):
        for i in range(64):
            sel[v, 64 * v + i, i] = 1.0
            sel[v, 64 * v + ((i + 32) % 64), 64 + i] = 1.0
    sel = sel.astype(_BF)

    # per-batch cos/sin stack [c; c; -s; s]
    inv_freq = 1.0 / (THETA ** (np.arange(0, DR, 2, dtype=np.float32) / DR))
    cs_b = []
    for b in range(B):
        ang = positions[b].astype(np.float32)[None, :] * inv_freq[:, None]
        c, s = np.cos(ang), np.sin(ang)
        cs_b.append(np.concatenate([c, c, -s, s], axis=0).astype(_BF))

    wqa_bf = wqa.astype(_BF)
    in_maps = []
    for core in range(8):
        b, j = core // 4, core % 4
        hs = [4 * (core % 4) + i for i in range(HPC)]
        # wqb per head-group: [pass x4 | rot per head x4]
        cols = [wqb[:, h * DQK : h * DQK + DN] for h in hs]
        for h in hs:
            cols.append(wqb[:, h * DQK + DN : (h + 1) * DQK])
        wqb_hg = np.concatenate(cols, axis=1).astype(_BF)
        wkvk_hg = np.concatenate(
            [wkvb[:, h * (DN + DV) : h * (DN + DV) + DN] for h in hs], axis=1
        ).astype(_BF)
        wkvv_hg = np.concatenate(
            [wkvb[:, h * (DN + DV) + DN : (h + 1) * (DN + DV)] for h in hs], axis=1
        ).astype(_BF)
        wo_hg = wo[hs[0] * DV : (hs[-1] + 1) * DV, :].astype(_BF)
        xTb = np.ascontiguousarray(
            x[b, NB * j : NB * (j + 1), :].T).astype(_BF)
        in_maps.append({
            "xT": xTb,
            "wqa": wqa_bf,
            "wkva": wkva_aug,
            "wqb": wqb_hg,
            "wkvk": wkvk_hg,
            "wkvv": wkvv_hg,
            "wo": wo_hg,
            "cs": cs_b[b],
            "cso": np.ascontiguousarray(cs_b[b][:, NB * j : NB * (j + 1)]),
            "maskp": maskp,
            "eye2": eye2,
            "sel": sel,
        })
    return in_maps


def _run(inputs, trace=False, trace_kwargs=None):
    from concourse.bass_utils import run_bass_kernel_spmd

    nc = _get_nc()
    in_maps = _prep_inputs(**inputs)
    res = run_bass_kernel_spmd(
        nc, in_maps, list(range(8)), trace=trace,
        trace_kwargs=trace_kwargs or {},
    )
    outs = np.zeros((B, T, HID), np.float32)
    for core in range(8):
        outs[core // 4] += res.results[core]["out"]
    return outs, res


def kernel(**inputs) -> np.ndarray:
    out, _ = _run(inputs)
    return out


# revision 15
# speedup vs baseline: 1.0809x; 1.0390x over previous
"""DeepseekV3 attention on 8 TRN2 NeuronCores.

Sharding: phase 1 token-sharded latent projections (8 blocks of 512 tokens),
AllGather of latents within each 4-core batch group, phase 2 head-sharded
(4 heads per core) attention + partial o_proj; host sums the 4 partials
per batch. All matmuls bf16 with fp32 PSUM accumulation.

v4: kv latents (+ k rope) AllGathered first; batched single-DMA AG stores;
PE warmup matmuls during the initial weight DMA; attention on 256-token query
blocks with two heads interleaved per pipeline (hides ACT/DVE handoff
latency); sum-of-exp accumulated on the PE; o_proj evacuation on ScalarE.
"""
import numpy as np
import ml_dtypes

import concourse.bacc as bacc
import concourse.mybir as mybir
import concourse.tile as tile

B, T, HID = 2, 2048, 2048
NH = 16
QLR, KVLR = 1536, 512
DN, DR = 128, 64
DQK, DV = DN + DR, 128
EPS = 1e-6
THETA = 10000.0
SCALE = DQK ** -0.5

NB = 512          # tokens per phase-1 block
HPC = 4           # heads per core in phase 2
HB = NB // 2      # q-latent AllGather half (256 tokens)
KVF = 4 * NB + HB  # 2304 free elems per partition in kv AG buffer

f32 = mybir.dt.float32
bf16 = mybir.dt.bfloat16
Exp = mybir.ActivationFunctionType.Exp
Sqrt = mybir.ActivationFunctionType.Sqrt
Square = mybir.ActivationFunctionType.Square
Copy = mybir.ActivationFunctionType.Copy

_BF = ml_dtypes.bfloat16


def _build():
    nc = bacc.Bacc(None, num_devices=8)

    # ---- per-core inputs ----
    xT = nc.declare_dram_parameter("xT", [HID, NB], bf16, isOutput=False)
    wqa = nc.declare_dram_parameter("wqa", [HID, QLR], bf16, isOutput=False)
    wkva = nc.declare_dram_parameter("wkva", [HID, KVLR + 2 * DR], bf16, isOutput=False)
    wqb = nc.declare_dram_parameter("wqb", [QLR, 768], bf16, isOutput=False)
    sel = nc.declare_dram_parameter("sel", [2, 128, 128], bf16, isOutput=False)
    wkvk = nc.declare_dram_parameter("wkvk", [KVLR, 512], bf16, isOutput=False)
    wkvv = nc.declare_dram_parameter("wkvv", [KVLR, 512], bf16, isOutput=False)
    wo = nc.declare_dram_parameter("wo", [HPC * DV, HID], bf16, isOutput=False)
    cs = nc.declare_dram_parameter("cs", [128, T], bf16, isOutput=False)  # [c;c;-s;s]
    cso = nc.declare_dram_parameter("cso", [128, NB], bf16, isOutput=False)  # own block
    maskp = nc.declare_dram_parameter("maskp", [128, 128], bf16, isOutput=False)
    eye2 = nc.declare_dram_parameter("eye2", [128, 64], bf16, isOutput=False)
    out = nc.declare_dram_parameter("out", [T, HID], f32, isOutput=True)

    # AG buffers: [128 partitions, free] so phase-2 reads are 1 big DMA each.
    # kv: 4 latent m-tiles of 512 cols + roped krot packed [64,512]->[128,256]
    ag_in_kv = nc.dram_tensor("ag_in_kv", [128, KVF], bf16)
    ag_out_kv = nc.dram_tensor("ag_out_kv", [4, 128, KVF], bf16)
    ag_in_q0 = nc.dram_tensor("ag_in_q0", [128, 12, HB], bf16)
    ag_out_q0 = nc.dram_tensor("ag_out_q0", [4, 128, 12, HB], bf16)
    ag_in_q1 = nc.dram_tensor("ag_in_q1", [128, 12, HB], bf16)
    ag_out_q1 = nc.dram_tensor("ag_out_q1", [4, 128, 12, HB], bf16)

    with tile.TileContext(nc) as tc:
        # ============ phase 1: latents for own 512-token block ============
        with (
            tc.tile_pool(name="p1", bufs=1) as p1,
            tc.tile_pool(name="p1w", bufs=2) as p1w,
            tc.tile_pool(name="ps1", bufs=2, space="PSUM") as ps1,
            tc.tile_pool(name="ps1r", bufs=1, space="PSUM") as ps1r,
            tc.tile_pool(name="ps1acc", bufs=2, space="PSUM") as ps1acc,
        ):
            ones = p1.tile([128, 128], bf16, tag="ones")
            nc.vector.memset(ones[:], 1.0)
            # PE warmup: ~4us of junk matmuls while the input DMAs land, so
            # the HAM clock-gate is already at 8/8 when real matmuls start
            wps = ps1r.tile([128, 64], f32, tag="warm")
            for _ in range(48):
                nc.tensor.matmul(wps[:], ones[:], ones[:, 0:64],
                                 start=True, stop=True)

            xt = p1.tile([128, 16, NB], bf16, tag="xt")
            wkvat = p1.tile([128, 16, KVLR + 2 * DR], bf16, tag="wkvat")
            wqat = p1.tile([128, 16, QLR], bf16, tag="wqat")
            for k in range(16):
                nc.sync.dma_start(xt[:, k], xT[128 * k : 128 * (k + 1), :])
                nc.sync.dma_start(wkvat[:, k], wkva[128 * k : 128 * (k + 1), :])
            csot = p1.tile([128, NB], bf16, tag="csot")
            nc.sync.dma_start(csot[:], cso[:])
            eyet = p1.tile([128, 64], bf16, tag="eyet")
            nc.sync.dma_start(eyet[:], eye2[:])
            for k in range(16):
                nc.sync.dma_start(wqat[:, k], wqa[128 * k : 128 * (k + 1), :])

            def compute_inv(ssq, d):
                w = ssq.shape[1]
                mt_ = p1w.tile([128, NB], f32, tag="rmst", name="rmst")
                nc.vector.tensor_scalar(
                    mt_[:, :w], ssq[:], 1.0 / d, EPS,
                    mybir.AluOpType.mult, mybir.AluOpType.add,
                )
                rms = p1w.tile([128, NB], f32, tag="rms", name="rms")
                nc.scalar.activation(rms[:, :w], mt_[:, :w], Sqrt)
                inv = p1w.tile([128, NB], f32, tag="inv", name="inv")
                nc.vector.reciprocal(inv[:, :w], rms[:, :w])
                return inv

            # ---- ckv^T: m 0..3 kv_lat (normed), m 4 = rope(k_rot) ----
            kvlat = p1.tile([128, 4, NB], f32, tag="kvlat")
            ssq_kv = ps1acc.tile([128, NB], f32, tag="ssq")
            for m in range(5):
                ps = ps1.tile([128, NB], f32, tag="p1ps")
                for k in range(16):
                    nc.tensor.matmul(
                        ps[:], wkvat[:, k, 128 * m : 128 * (m + 1)], xt[:, k],
                        start=(k == 0), stop=(k == 15),
                    )
                if m < 4:
                    sq = p1w.tile([128, NB], bf16, tag="sq")
                    nc.scalar.activation(sq[:], ps[:], Square)
                    nc.vector.tensor_copy(kvlat[:, m], ps[:])
                    nc.tensor.matmul(ssq_kv[:], ones[:], sq[:],
                                     start=(m == 0), stop=(m == 3))
                else:
                    # rope k: ps rows = [kr; swap(kr)]; cso = [c;c;-s;s]
                    tt = p1w.tile([128, NB], bf16, tag="kropet")
                    nc.vector.tensor_mul(tt[:], ps[:], csot[:])
                    pr = ps1r.tile([64, NB], f32, tag="kropeps")
                    nc.tensor.matmul(pr[:], eyet[:], tt[:], start=True, stop=True)
                    rot = p1w.tile([64, NB], bf16, tag="rot")
                    nc.vector.tensor_copy(rot[:], pr[:])
                    nc.gpsimd.dma_start(
                        ag_in_kv[0:64, 4 * NB : 4 * NB + HB], rot[:, 0:HB])
                    nc.gpsimd.dma_start(
                        ag_in_kv[64:128, 4 * NB : 4 * NB + HB], rot[:, HB:NB])

            inv = compute_inv(ssq_kv, KVLR)
            nkv = p1w.tile([128, 4 * NB], bf16, tag="normkv")
            for m in range(4):
                nc.vector.tensor_mul(
                    nkv[:, NB * m : NB * (m + 1)], kvlat[:, m], inv[:, :NB])
            nc.gpsimd.dma_start(ag_in_kv[:, 0 : 4 * NB], nkv[:])
            nc.gpsimd.collective_compute(
                "AllGather", mybir.AluOpType.bypass,
                replica_groups=[[0, 1, 2, 3], [4, 5, 6, 7]],
                ins=[ag_in_kv[:]], outs=[ag_out_kv[:]],
            )

            # ---- q_lat^T in two column halves so AG_q0 starts early ----
            for half, (agi, ago_) in enumerate(
                ((ag_in_q0, ag_out_q0), (ag_in_q1, ag_out_q1))
            ):
                c0 = half * HB
                qlat = p1.tile([128, 12, HB], f32, tag="qlat", name="qlat")
                ssq_q = ps1acc.tile([128, HB], f32, tag="ssq", name="ssq_q")
                for m in range(12):
                    ps = ps1.tile([128, HB], f32, tag="p1psq", name="p1psq")
                    for k in range(16):
                        nc.tensor.matmul(
                            ps[:], wqat[:, k, 128 * m : 128 * (m + 1)],
                            xt[:, k, c0 : c0 + HB],
                            start=(k == 0), stop=(k == 15),
                        )
                    nc.vector.tensor_copy(qlat[:, m], ps[:])
                    sq = p1w.tile([128, HB], bf16, tag="sqq", name="sqq")
                    nc.vector.tensor_mul(sq[:], qlat[:, m], qlat[:, m])
                    nc.tensor.matmul(ssq_q[:], ones[:], sq[:],
                                     start=(m == 0), stop=(m == 11))
                inv = compute_inv(ssq_q, QLR)
                nq = p1w.tile([128, 12, HB], bf16, tag="normq", name="normq")
                for m in range(12):
                    nc.vector.tensor_mul(nq[:, m], qlat[:, m], inv[:, :HB])
                nc.gpsimd.dma_start(agi[:], nq[:])
                nc.gpsimd.collective_compute(
                    "AllGather", mybir.AluOpType.bypass,
                    replica_groups=[[0, 1, 2, 3], [4, 5, 6, 7]],
                    ins=[agi[:]], outs=[ago_[:]],
                )

        # ============ phase 2: 4 heads, full batch ============
        with tc.tile_pool(name="p2", bufs=1) as p2:
            qTp = p2.tile([128, 4, 4, NB], bf16, tag="qTp")      # pass, per head m
            qrot = [
                p2.tile([128, T], bf16, tag=f"qrot{h}", name=f"qrot{h}")
                for h in range(HPC)
            ]
            krotT2 = p2.tile([128, T], bf16, tag="krotT2")       # [kr;kr] stacked
            kpT = p2.tile([128, 4, 4, NB], bf16, tag="kpT")      # [d, head, r, t]
            vT = p2.tile([128, 16, 512], bf16, tag="vT")         # [t, t-tile, dv]
            attnT = p2.tile([128, 4, 8, HB], bf16, tag="attnT")  # [dv, head, qb, t]
            cst = p2.tile([128, T], bf16, tag="cst")
            nc.sync.dma_start(cst[:], cs[:])
            maskt = p2.tile([128, 128], bf16, tag="maskt")
            nc.sync.dma_start(maskt[:], maskp[:])
            onesb = p2.tile([128, 128], bf16, tag="onesb")
            nc.vector.memset(onesb[:], 1.0)
            wot = p2.tile([128, 4, HID], bf16, tag="wot")
            for k in range(4):
                nc.sync.dma_start(wot[:, k], wo[128 * k : 128 * (k + 1), :])

            # ---- 2b: k_pass^T, V (needs AG_kv) ----
            with (
                tc.tile_pool(name="p2b", bufs=1) as p2b,
                tc.tile_pool(name="p2bl", bufs=2) as p2bl,
                tc.tile_pool(name="ps2b", bufs=2, space="PSUM") as ps2b,
            ):
                wkkt = p2b.tile([128, 4, 512], bf16, tag="wkkt")
                wkvt = p2b.tile([128, 4, 512], bf16, tag="wkvt")
                for k in range(4):
                    nc.sync.dma_start(wkkt[:, k], wkvk[128 * k : 128 * (k + 1), :])
                    nc.sync.dma_start(wkvt[:, k], wkvv[128 * k : 128 * (k + 1), :])
                for r in range(4):
                    kvl = p2bl.tile([128, 4, NB], bf16, tag="kvl", name="kvl")
                    nc.gpsimd.dma_start(kvl[:], ag_out_kv[r, :, 0 : 4 * NB])
                    for dup in (0, 1):
                        nc.gpsimd.dma_start(
                            krotT2[64 * dup : 64 * dup + 64,
                                   NB * r : NB * r + HB],
                            ag_out_kv[r, 0:64, 4 * NB : 4 * NB + HB])
                        nc.gpsimd.dma_start(
                            krotT2[64 * dup : 64 * dup + 64,
                                   NB * r + HB : NB * (r + 1)],
                            ag_out_kv[r, 64:128, 4 * NB : 4 * NB + HB])
                    # k_pass^T for 4 heads
                    for m in range(4):
                        ps = ps2b.tile([128, NB], f32, tag="k2ps")
                        for k in range(4):
                            nc.tensor.matmul(
                                ps[:], wkkt[:, k, 128 * m : 128 * (m + 1)],
                                kvl[:, k], start=(k == 0), stop=(k == 3),
                            )
                        nc.vector.tensor_copy(kpT[:, m, r], ps[:])
                    # V token-major
                    for s in range(4):
                        ps = ps2b.tile([128, 512], f32, tag="v2ps")
                        for k in range(4):
                            nc.tensor.matmul(
                                ps[:], kvl[:, k, 128 * s : 128 * (s + 1)],
                                wkvt[:, k], start=(k == 0), stop=(k == 3),
                            )
                        nc.vector.tensor_copy(vT[:, 4 * r + s], ps[:])

            # ---- 2a + attention + o_proj, pipelined per (half, r) ----
            # PSUM: psS 4 (q2ps/scps/selps/ops shared) + psSum 2 + psB 2 = 8
            with (
                tc.tile_pool(name="p2a", bufs=1) as p2a,
                tc.tile_pool(name="p2as", bufs=2) as p2as,
                tc.tile_pool(name="p2d", bufs=5) as p2d,
                tc.tile_pool(name="p2dn", bufs=3) as p2dn,
                tc.tile_pool(name="p2eo", bufs=3) as p2eo,
                tc.tile_pool(name="psS", bufs=6, space="PSUM") as psS,
                tc.tile_pool(name="psB", bufs=2, space="PSUM") as psB,
            ):
                wqbt = p2a.tile([128, 12, 768], bf16, tag="wqbt")
                selt = p2a.tile([128, 2, 128], bf16, tag="selt")
                for v in range(2):
                    nc.sync.dma_start(selt[:, v], sel[v])
                for k in range(12):
                    nc.sync.dma_start(wqbt[:, k], wqb[128 * k : 128 * (k + 1), :])
                qrw = p2a.tile([128, 2, 4, NB], bf16, tag="qrw")  # raw rot pairs

                def attention_pair(h2, qb):
                    nkt = 2 * qb + 2
                    aps, eac = {}, {}
                    for h in h2:
                        aps[h] = psB.tile([128, HB], f32, tag="attn_ps",
                                          name=f"aps{h}")
                        eac[h] = p2dn.tile([128, HB], bf16, tag="eacc",
                                           name=f"eac{h}")
                    pend = []

                    def emit_pv(h, pe, pk, off):
                        nc.tensor.matmul(
                            aps[h][:, off:HB], vT[:, pk, 128 * h : 128 * (h + 1)],
                            pe[:, off:HB], start=(pk == 0), stop=(pk == nkt - 1),
                        )

                    for kt in range(nkt):
                        m = kt - 2 * qb
                        off = 128 * m if m > 0 else 0
                        rk, sl = kt // 4, 128 * (kt % 4)
                        for h in h2:
                            scp = psS.tile([128, HB], f32, tag="big",
                                           name="scps")
                            nc.tensor.matmul(
                                scp[:, off:HB], kpT[:, h, rk, sl : sl + 128],
                                qTp[:, h, qb // 2,
                                    (qb % 2) * HB + off : (qb % 2 + 1) * HB],
                                start=True, stop=False,
                            )
                            nc.tensor.matmul(
                                scp[:, off:HB],
                                krotT2[:, 128 * kt : 128 * kt + 128],
                                qrot[h][:, HB * qb + off : HB * (qb + 1)],
                                start=False, stop=True,
                            )
                            et = p2d.tile([128, HB], bf16, tag="expT")
                            nc.scalar.activation(
                                et[:, off:HB], scp[:, off:HB], Exp, scale=SCALE
                            )
                            if m >= 0:
                                nc.vector.tensor_mul(
                                    et[:, off : off + 128],
                                    et[:, off : off + 128], maskt[:],
                                )
                            if kt == 0:
                                nc.vector.tensor_copy(eac[h][:], et[:])
                            else:
                                nc.vector.tensor_add(
                                    eac[h][:, off:HB], eac[h][:, off:HB],
                                    et[:, off:HB],
                                )
                            if len(pend) == 2:
                                emit_pv(*pend.pop(0))
                            pend.append((h, et, kt, off))
                    for p_ in pend:
                        emit_pv(*p_)
                    for h in h2:
                        sps = psS.tile([128, HB], f32, tag="big",
                                       name=f"sps{h}")
                        nc.tensor.matmul(sps[:], onesb[:], eac[h][:],
                                         start=True, stop=True)
                        rec = p2dn.tile([128, HB], f32, tag="rec",
                                        name=f"rec{h}")
                        nc.vector.reciprocal(rec[:], sps[:])
                        nc.vector.tensor_mul(attnT[:, h, qb], aps[h][:], rec[:])

                for c, agq in ((0, ag_out_q0), (1, ag_out_q1)):
                    for r in range(4):
                        # scalar HWDGE ring: keeps these AG_q-gated loads out
                        # of the sync-engine DMA FIFO (head-of-line blocking)
                        qlt = p2as.tile([128, 12, HB], bf16, tag="qlt")
                        nc.gpsimd.dma_start(qlt[:], agq[r])
                        for m in range(6):
                            ps = psS.tile([128, HB], f32, tag="big", name="q2ps")
                            for k in range(12):
                                nc.tensor.matmul(
                                    ps[:], wqbt[:, k, 128 * m : 128 * (m + 1)],
                                    qlt[:, k], start=(k == 0), stop=(k == 11),
                                )
                            dst = qTp[:, m, r] if m < 4 else qrw[:, m - 4, r]
                            nc.any.tensor_copy(
                                dst[:, c * HB : (c + 1) * HB], ps[:]
                            )
                        # rope q: sel-matmul + CS-mul; the [rot;swap] pair-
                        # combine folds into the score matmul via krotT2
                        for h in range(HPC):
                            sp = psS.tile([128, HB], f32, tag="big",
                                          name="selps")
                            nc.tensor.matmul(
                                sp[:], selt[:, h % 2],
                                qrw[:, h // 2, r, c * HB : (c + 1) * HB],
                                start=True, stop=True)
                            nc.vector.tensor_mul(
                                qrot[h][:, NB * r + c * HB : NB * r + (c + 1) * HB],
                                sp[:],
                                cst[:, NB * r + c * HB : NB * r + (c + 1) * HB],
                            )
                        qb = 2 * r + c
                        attention_pair((0, 1), qb)
                        attention_pair((2, 3), qb)
                        # o_proj partial for this query block
                        for half2 in range(2):
                            tm = 4 * r + 2 * c + half2
                            s = 128 * half2
                            for n in range(4):
                                ps = psS.tile([128, 512], f32, tag="big",
                                              name="ops")
                                for k in range(4):
                                    nc.tensor.matmul(
                                        ps[:], attnT[:, k, qb, s : s + 128],
                                        wot[:, k, 512 * n : 512 * (n + 1)],
                                        start=(k == 0), stop=(k == 3),
                                    )
                                ot = p2eo.tile([128, 512], f32, tag="oT")
                                nc.any.tensor_copy(ot[:], ps[:])
                                nc.sync.dma_start(
                                    out[128 * tm : 128 * (tm + 1),
                                        512 * n : 512 * (n + 1)], ot[:],
                                )

    nc.finalize()
    return nc


_NC = None


def _get_nc():
    global _NC
    if _NC is None:
        _NC = _build()
    return _NC


def _prep_inputs(x, attention_mask, positions, wqa, qa_scale, wqb, wkva,
                 kva_scale, wkvb, wo):
    x = np.asarray(x, np.float32)
    positions = np.asarray(positions)
    wqa = np.asarray(wqa, np.float32)
    wqb = np.asarray(wqb, np.float32) * np.asarray(qa_scale, np.float32)[:, None]
    wkva = np.asarray(wkva, np.float32)
    wkvb = np.asarray(wkvb, np.float32) * np.asarray(kva_scale, np.float32)[:, None]
    wo = np.asarray(wo, np.float32)

    # wkva augmented with swapped-rot columns
    kr = wkva[:, KVLR:]
    wkva_aug = np.concatenate(
        [wkva[:, :KVLR], kr, kr[:, DR // 2 :], kr[:, : DR // 2]], axis=1
    ).astype(_BF)

    # triu mask for the 128-col diagonal window: mask[r, c] = c >= r
    rr = np.arange(128)[:, None]
    cc = np.arange(128)[None, :]
    maskp = (cc >= rr).astype(_BF)

    eye2 = np.concatenate([np.eye(64), np.eye(64)], axis=0).astype(_BF)

    # sel[v]: out rows [0:64]=src rows [64v:64v+64]; [64:128]=32-swapped copy
    sel = np.zeros((2, 128, 128), np.float32)
    for v in range(2):
        for i in range(64):
            sel[v, 64 * v + i, i] = 1.0
            sel[v, 64 * v + ((i + 32) % 64), 64 + i] = 1.0
    sel = sel.astype(_BF)

    # per-batch cos/sin stack [c; c; -s; s]
    inv_freq = 1.0 / (THETA ** (np.arange(0, DR, 2, dtype=np.float32) / DR))
    cs_b = []
    for b in range(B):
        ang = positions[b].astype(np.float32)[None, :] * inv_freq[:, None]
        c, s = np.cos(ang), np.sin(ang)
        cs_b.append(np.concatenate([c, c, -s, s], axis=0).astype(_BF))

    wqa_bf = wqa.astype(_BF)
    in_maps = []
    for core in range(8):
        b, j = core // 4, core % 4
        hs = [4 * (core % 4) + i for i in range(HPC)]
        # wqb per head-group: [pass x4 | rot per head x4]
        cols = [wqb[:, h * DQK : h * DQK + DN] for h in hs]
        for h in hs:
            cols.append(wqb[:, h * DQK + DN : (h + 1) * DQK])
        wqb_hg = np.concatenate(cols, axis=1).astype(_BF)
        wkvk_hg = np.concatenate(
            [wkvb[:, h * (DN + DV) : h * (DN + DV) + DN] for h in hs], axis=1
        ).astype(_BF)
        wkvv_hg = np.concatenate(
            [wkvb[:, h * (DN + DV) + DN : (h + 1) * (DN + DV)] for h in hs], axis=1
        ).astype(_BF)
        wo_hg = wo[hs[0] * DV : (hs[-1] + 1) * DV, :].astype(_BF)
        xTb = np.ascontiguousarray(
            x[b, NB * j : NB * (j + 1), :].T).astype(_BF)
        in_maps.append({
            "xT": xTb,
            "wqa": wqa_bf,
            "wkva": wkva_aug,
            "wqb": wqb_hg,
            "wkvk": wkvk_hg,
            "wkvv": wkvv_hg,
            "wo": wo_hg,
            "cs": cs_b[b],
            "cso": np.ascontiguousarray(cs_b[b][:, NB * j : NB * (j + 1)]),
            "maskp": maskp,
            "eye2": eye2,
            "sel": sel,
        })
    return in_maps


def _run(inputs, trace=False, trace_kwargs=None):
    from concourse.bass_utils import run_bass_kernel_spmd

    nc = _get_nc()
    in_maps = _prep_inputs(**inputs)
    res = run_bass_kernel_spmd(
        nc, in_maps, list(range(8)), trace=trace,
        trace_kwargs=trace_kwargs or {},
    )
    outs = np.zeros((B, T, HID), np.float32)
    for core in range(8):
        outs[core // 4] += res.results[core]["out"]
    return outs, res


def kernel(**inputs) -> np.ndarray:
    out, _ = _run(inputs)
    return out


# revision 22
# speedup vs baseline: 1.1241x; 1.0400x over previous
"""DeepseekV3 attention on 8 TRN2 NeuronCores.

Sharding: phase 1 token-sharded latent projections (8 blocks of 512 tokens),
AllGather of latents within each 4-core batch group, phase 2 head-sharded
(4 heads per core) attention + partial o_proj; host sums the 4 partials
per batch. All matmuls bf16 with fp32 PSUM accumulation.

Key scheduling choices (each verified against NTFF traces):
- kv latents (+ roped k) computed and AllGathered first, then the two q-latent
  halves, so the three serialized collectives overlap phase-2 compute.
- All phase-2 weights DMA'd up front (after phase-1 inputs): once the
  AllGathers start they saturate HBM and mid-kernel loads crawl.
- Collective-gated loads go through SWDGE (gpsimd) so Tile's shared HWDGE
  completion lanes stay monotone (no head-of-line blocking of 2a/2b matmuls).
- Batched single-DMA AllGather stores; PE warmup matmuls at t=0 (HAM 8/8).
- Attention per 512-token query block with two heads interleaved per kt step
  (hides ACT/DVE handoff latency); q-rope pair-combine folded into the score
  matmul via duplicated krotT rows; DVE reciprocal (ACT Ln/Exp thrashes
  activation-table loads); o_proj interleaved per block, out stores on sync.
"""
import numpy as np
import ml_dtypes

import concourse.bacc as bacc
import concourse.mybir as mybir
import concourse.tile as tile

B, T, HID = 2, 2048, 2048
NH = 16
QLR, KVLR = 1536, 512
DN, DR = 128, 64
DQK, DV = DN + DR, 128
EPS = 1e-6
THETA = 10000.0
SCALE = DQK ** -0.5

NB = 512          # tokens per phase-1 block
HPC = 4           # heads per core in phase 2
HB = NB // 2      # q-latent AllGather half (256 tokens)
KVF = 4 * NB + HB  # 2304 free elems per partition in kv AG buffer

f32 = mybir.dt.float32
bf16 = mybir.dt.bfloat16
Exp = mybir.ActivationFunctionType.Exp
Sqrt = mybir.ActivationFunctionType.Sqrt
Square = mybir.ActivationFunctionType.Square
Copy = mybir.ActivationFunctionType.Copy

_BF = ml_dtypes.bfloat16


def _build():
    nc = bacc.Bacc(None, num_devices=8)

    # ---- per-core inputs ----
    xT = nc.declare_dram_parameter("xT", [HID, NB], bf16, isOutput=False)
    wqa = nc.declare_dram_parameter("wqa", [HID, QLR], bf16, isOutput=False)
    wkva = nc.declare_dram_parameter("wkva", [HID, KVLR + 2 * DR], bf16, isOutput=False)
    wqb = nc.declare_dram_parameter("wqb", [QLR, 768], bf16, isOutput=False)
    sel = nc.declare_dram_parameter("sel", [2, 128, 128], bf16, isOutput=False)
    wkvk = nc.declare_dram_parameter("wkvk", [KVLR, 512], bf16, isOutput=False)
    wkvv = nc.declare_dram_parameter("wkvv", [KVLR, 512], bf16, isOutput=False)
    wo = nc.declare_dram_parameter("wo", [HPC * DV, HID], bf16, isOutput=False)
    cs = nc.declare_dram_parameter("cs", [128, T], bf16, isOutput=False)  # [c;c;-s;s]
    cso = nc.declare_dram_parameter("cso", [128, NB], bf16, isOutput=False)  # own block
    maskp = nc.declare_dram_parameter("maskp", [128, 128], bf16, isOutput=False)
    eye2 = nc.declare_dram_parameter("eye2", [128, 64], bf16, isOutput=False)
    out = nc.declare_dram_parameter("out", [T, HID], f32, isOutput=True)

    # AG buffers: [128 partitions, free] so phase-2 reads are 1 big DMA each.
    # kv: 4 latent m-tiles of 512 cols + roped krot packed [64,512]->[128,256]
    ag_in_kv = nc.dram_tensor("ag_in_kv", [128, KVF], bf16)
    ag_out_kv = nc.dram_tensor("ag_out_kv", [4, 128, KVF], bf16)
    ag_in_q0 = nc.dram_tensor("ag_in_q0", [128, 12, HB], bf16)
    ag_out_q0 = nc.dram_tensor("ag_out_q0", [4, 128, 12, HB], bf16)
    ag_in_q1 = nc.dram_tensor("ag_in_q1", [128, 12, HB], bf16)
    ag_out_q1 = nc.dram_tensor("ag_out_q1", [4, 128, 12, HB], bf16)

    with tile.TileContext(nc) as tc:
        # phase-2 weights preloaded up front: once the AllGathers start they
        # saturate HBM and mid-kernel weight loads crawl
        p2w = tc.tile_pool(name="p2w", bufs=1).__enter__()
        wqbt = p2w.tile([128, 12, 768], bf16, tag="wqbt")
        selt = p2w.tile([128, 2, 128], bf16, tag="selt")
        wkkt = p2w.tile([128, 4, 512], bf16, tag="wkkt")
        wkvt = p2w.tile([128, 4, 512], bf16, tag="wkvt")
        wot = p2w.tile([128, 4, HID], bf16, tag="wot")
        cst = p2w.tile([128, T], bf16, tag="cst")
        maskt = p2w.tile([128, 128], bf16, tag="maskt")
        onesb = p2w.tile([128, 128], bf16, tag="onesb")
        for k in range(4):
            nc.sync.dma_start(wkkt[:, k], wkvk[128 * k : 128 * (k + 1), :])
            nc.sync.dma_start(wkvt[:, k], wkvv[128 * k : 128 * (k + 1), :])
        for k in range(12):
            nc.sync.dma_start(wqbt[:, k], wqb[128 * k : 128 * (k + 1), :])
        for k in range(4):
            nc.sync.dma_start(wot[:, k], wo[128 * k : 128 * (k + 1), :])
        nc.sync.dma_start(cst[:], cs[:])
        nc.sync.dma_start(maskt[:], maskp[:])
        for v in range(2):
            nc.sync.dma_start(selt[:, v], sel[v])
        nc.vector.memset(onesb[:], 1.0)

        # ============ phase 1: latents for own 512-token block ============
        with (
            tc.tile_pool(name="p1", bufs=1) as p1,
            tc.tile_pool(name="p1w", bufs=2) as p1w,
            tc.tile_pool(name="ps1", bufs=2, space="PSUM") as ps1,
            tc.tile_pool(name="ps1r", bufs=1, space="PSUM") as ps1r,
            tc.tile_pool(name="ps1acc", bufs=2, space="PSUM") as ps1acc,
        ):
            ones = p1.tile([128, 128], bf16, tag="ones")
            nc.vector.memset(ones[:], 1.0)
            # PE warmup: ~4us of junk matmuls while the input DMAs land, so
            # the HAM clock-gate is already at 8/8 when real matmuls start
            wps = ps1r.tile([128, 64], f32, tag="warm")
            for _ in range(48):
                nc.tensor.matmul(wps[:], ones[:], ones[:, 0:64],
                                 start=True, stop=True)

            xt = p1.tile([128, 16, NB], bf16, tag="xt")
            wkvat = p1.tile([128, 16, KVLR + 2 * DR], bf16, tag="wkvat")
            wqat = p1.tile([128, 16, QLR], bf16, tag="wqat")
            for k in range(16):
                nc.sync.dma_start(xt[:, k], xT[128 * k : 128 * (k + 1), :])
                nc.sync.dma_start(wkvat[:, k], wkva[128 * k : 128 * (k + 1), :])
            csot = p1.tile([128, NB], bf16, tag="csot")
            nc.sync.dma_start(csot[:], cso[:])
            eyet = p1.tile([128, 64], bf16, tag="eyet")
            nc.sync.dma_start(eyet[:], eye2[:])
            for k in range(16):
                nc.sync.dma_start(wqat[:, k], wqa[128 * k : 128 * (k + 1), :])

            def compute_inv(ssq, d):
                w = ssq.shape[1]
                mt_ = p1w.tile([128, NB], f32, tag="rmst", name="rmst")
                nc.vector.tensor_scalar(
                    mt_[:, :w], ssq[:], 1.0 / d, EPS,
                    mybir.AluOpType.mult, mybir.AluOpType.add,
                )
                rms = p1w.tile([128, NB], f32, tag="rms", name="rms")
                nc.scalar.activation(rms[:, :w], mt_[:, :w], Sqrt)
                inv = p1w.tile([128, NB], f32, tag="inv", name="inv")
                nc.vector.reciprocal(inv[:, :w], rms[:, :w])
                return inv

            # ---- ckv^T: m 0..3 kv_lat (normed), m 4 = rope(k_rot) ----
            kvlat = p1.tile([128, 4, NB], f32, tag="kvlat")
            ssq_kv = ps1acc.tile([128, NB], f32, tag="ssq")
            for m in range(5):
                ps = ps1.tile([128, NB], f32, tag="p1ps")
                for k in range(16):
                    nc.tensor.matmul(
                        ps[:], wkvat[:, k, 128 * m : 128 * (m + 1)], xt[:, k],
                        start=(k == 0), stop=(k == 15),
                    )
                if m < 4:
                    sq = p1w.tile([128, NB], bf16, tag="sq")
                    nc.scalar.activation(sq[:], ps[:], Square)
                    nc.vector.tensor_copy(kvlat[:, m], ps[:])
                    nc.tensor.matmul(ssq_kv[:], ones[:], sq[:],
                                     start=(m == 0), stop=(m == 3))
                else:
                    # rope k: ps rows = [kr; swap(kr)]; cso = [c;c;-s;s]
                    tt = p1w.tile([128, NB], bf16, tag="kropet")
                    nc.vector.tensor_mul(tt[:], ps[:], csot[:])
                    pr = ps1r.tile([64, NB], f32, tag="kropeps")
                    nc.tensor.matmul(pr[:], eyet[:], tt[:], start=True, stop=True)
                    rot = p1w.tile([64, NB], bf16, tag="rot")
                    nc.vector.tensor_copy(rot[:], pr[:])
                    nc.gpsimd.dma_start(
                        ag_in_kv[0:64, 4 * NB : 4 * NB + HB], rot[:, 0:HB])
                    nc.gpsimd.dma_start(
                        ag_in_kv[64:128, 4 * NB : 4 * NB + HB], rot[:, HB:NB])

            inv = compute_inv(ssq_kv, KVLR)
            nkv = p1w.tile([128, 4 * NB], bf16, tag="normkv")
            for m in range(4):
                nc.vector.tensor_mul(
                    nkv[:, NB * m : NB * (m + 1)], kvlat[:, m], inv[:, :NB])
            nc.gpsimd.dma_start(ag_in_kv[:, 0 : 4 * NB], nkv[:])
            nc.gpsimd.collective_compute(
                "AllGather", mybir.AluOpType.bypass,
                replica_groups=[[0, 1, 2, 3], [4, 5, 6, 7]],
                ins=[ag_in_kv[:]], outs=[ag_out_kv[:]],
            )

            # ---- q_lat^T in two column halves so AG_q0 starts early ----
            for half, (agi, ago_) in enumerate(
                ((ag_in_q0, ag_out_q0), (ag_in_q1, ag_out_q1))
            ):
                c0 = half * HB
                qlat = p1.tile([128, 12, HB], f32, tag="qlat", name="qlat")
                ssq_q = ps1acc.tile([128, HB], f32, tag="ssq", name="ssq_q")
                for m in range(12):
                    ps = ps1.tile([128, HB], f32, tag="p1psq", name="p1psq")
                    for k in range(16):
                        nc.tensor.matmul(
                            ps[:], wqat[:, k, 128 * m : 128 * (m + 1)],
                            xt[:, k, c0 : c0 + HB],
                            start=(k == 0), stop=(k == 15),
                        )
                    nc.vector.tensor_copy(qlat[:, m], ps[:])
                    sq = p1w.tile([128, HB], bf16, tag="sqq", name="sqq")
                    nc.vector.tensor_mul(sq[:], qlat[:, m], qlat[:, m])
                    nc.tensor.matmul(ssq_q[:], ones[:], sq[:],
                                     start=(m == 0), stop=(m == 11))
                inv = compute_inv(ssq_q, QLR)
                nq = p1w.tile([128, 12, HB], bf16, tag="normq", name="normq")
                for m in range(12):
                    nc.vector.tensor_mul(nq[:, m], qlat[:, m], inv[:, :HB])
                nc.gpsimd.dma_start(agi[:], nq[:])
                nc.gpsimd.collective_compute(
                    "AllGather", mybir.AluOpType.bypass,
                    replica_groups=[[0, 1, 2, 3], [4, 5, 6, 7]],
                    ins=[agi[:]], outs=[ago_[:]],
                )

        # ============ phase 2: 4 heads, full batch ============
        with tc.tile_pool(name="p2", bufs=1) as p2:
            qTp = p2.tile([128, 4, 4, NB], bf16, tag="qTp")      # pass, per head m
            qrot = [
                p2.tile([128, T], bf16, tag=f"qrot{h}", name=f"qrot{h}")
                for h in range(HPC)
            ]
            krotT2 = p2.tile([128, T], bf16, tag="krotT2")       # [kr;kr] stacked
            kpT = p2.tile([128, 4, 4, NB], bf16, tag="kpT")      # [d, head, r, t]
            vT = p2.tile([128, 16, 512], bf16, tag="vT")         # [t, t-tile, dv]
            attnT = p2.tile([128, 4, 8, HB], bf16, tag="attnT")  # [dv, head, qb, t]

            # ---- 2b: k_pass^T, V (needs AG_kv) ----
            with (
                tc.tile_pool(name="p2b", bufs=1) as p2b,
                tc.tile_pool(name="p2bl", bufs=2) as p2bl,
                tc.tile_pool(name="ps2b", bufs=2, space="PSUM") as ps2b,
            ):
                for r in range(4):
                    kvl = p2bl.tile([128, 4, NB], bf16, tag="kvl", name="kvl")
                    nc.gpsimd.dma_start(kvl[:], ag_out_kv[r, :, 0 : 4 * NB])
                    for dup in (0, 1):
                        nc.gpsimd.dma_start(
                            krotT2[64 * dup : 64 * dup + 64,
                                   NB * r : NB * r + HB],
                            ag_out_kv[r, 0:64, 4 * NB : 4 * NB + HB])
                        nc.gpsimd.dma_start(
                            krotT2[64 * dup : 64 * dup + 64,
                                   NB * r + HB : NB * (r + 1)],
                            ag_out_kv[r, 64:128, 4 * NB : 4 * NB + HB])
                    # k_pass^T for 4 heads
                    for m in range(4):
                        ps = ps2b.tile([128, NB], f32, tag="k2ps")
                        for k in range(4):
                            nc.tensor.matmul(
                                ps[:], wkkt[:, k, 128 * m : 128 * (m + 1)],
                                kvl[:, k], start=(k == 0), stop=(k == 3),
                            )
                        nc.vector.tensor_copy(kpT[:, m, r], ps[:])
                    # V token-major
                    for s in range(4):
                        ps = ps2b.tile([128, 512], f32, tag="v2ps")
                        for k in range(4):
                            nc.tensor.matmul(
                                ps[:], kvl[:, k, 128 * s : 128 * (s + 1)],
                                wkvt[:, k], start=(k == 0), stop=(k == 3),
                            )
                        nc.vector.tensor_copy(vT[:, 4 * r + s], ps[:])

            # ---- 2a + attention + o_proj, pipelined per (half, r) ----
            # PSUM: psS 4 (q2ps/scps/selps/ops shared) + psSum 2 + psB 2 = 8
            with (
                tc.tile_pool(name="p2a", bufs=1) as p2a,
                tc.tile_pool(name="p2as", bufs=2) as p2as,
                tc.tile_pool(name="p2d", bufs=5) as p2d,
                tc.tile_pool(name="p2dn", bufs=3) as p2dn,
                tc.tile_pool(name="p2eo", bufs=3) as p2eo,
                tc.tile_pool(name="psS", bufs=6, space="PSUM") as psS,
                tc.tile_pool(name="psB", bufs=2, space="PSUM") as psB,
            ):
                qrw = p2a.tile([128, 2, 4, NB], bf16, tag="qrw")  # raw rot pairs

                def attention_pair(h2, qb):
                    nkt = 2 * qb + 2
                    aps, eac = {}, {}
                    for h in h2:
                        aps[h] = psB.tile([128, HB], f32, tag="attn_ps",
                                          name=f"aps{h}")
                        eac[h] = p2dn.tile([128, HB], bf16, tag="eacc",
                                           name=f"eac{h}")
                    pend = []

                    def emit_pv(h, pe, pk, off):
                        nc.tensor.matmul(
                            aps[h][:, off:HB], vT[:, pk, 128 * h : 128 * (h + 1)],
                            pe[:, off:HB], start=(pk == 0), stop=(pk == nkt - 1),
                        )

                    for kt in range(nkt):
                        m = kt - 2 * qb
                        off = 128 * m if m > 0 else 0
                        rk, sl = kt // 4, 128 * (kt % 4)
                        for h in h2:
                            scp = psS.tile([128, HB], f32, tag="big",
                                           name="scps")
                            nc.tensor.matmul(
                                scp[:, off:HB], kpT[:, h, rk, sl : sl + 128],
                                qTp[:, h, qb // 2,
                                    (qb % 2) * HB + off : (qb % 2 + 1) * HB],
                                start=True, stop=False,
                            )
                            nc.tensor.matmul(
                                scp[:, off:HB],
                                krotT2[:, 128 * kt : 128 * kt + 128],
                                qrot[h][:, HB * qb + off : HB * (qb + 1)],
                                start=False, stop=True,
                            )
                            et = p2d.tile([128, HB], bf16, tag="expT")
                            nc.scalar.activation(
                                et[:, off:HB], scp[:, off:HB], Exp, scale=SCALE
                            )
                            if m >= 0:
                                nc.vector.tensor_mul(
                                    et[:, off : off + 128],
                                    et[:, off : off + 128], maskt[:],
                                )
                            if kt == 0:
                                nc.vector.tensor_copy(eac[h][:], et[:])
                            else:
                                nc.vector.tensor_add(
                                    eac[h][:, off:HB], eac[h][:, off:HB],
                                    et[:, off:HB],
                                )
                            if len(pend) == 2:
                                emit_pv(*pend.pop(0))
                            pend.append((h, et, kt, off))
                    for p_ in pend:
                        emit_pv(*p_)
                    for h in h2:
                        sps = psS.tile([128, HB], f32, tag="big",
                                       name=f"sps{h}")
                        nc.tensor.matmul(sps[:], onesb[:], eac[h][:],
                                         start=True, stop=True)
                        rec = p2dn.tile([128, HB], f32, tag="rec",
                                        name=f"rec{h}")
                        nc.vector.reciprocal(rec[:], sps[:])
                        nc.vector.tensor_mul(attnT[:, h, qb], aps[h][:], rec[:])

                for c, agq in ((0, ag_out_q0), (1, ag_out_q1)):
                    for r in range(4):
                        # scalar HWDGE ring: keeps these AG_q-gated loads out
                        # of the sync-engine DMA FIFO (head-of-line blocking)
                        qlt = p2as.tile([128, 12, HB], bf16, tag="qlt")
                        nc.gpsimd.dma_start(qlt[:], agq[r])
                        for m in range(6):
                            ps = psS.tile([128, HB], f32, tag="big", name="q2ps")
                            for k in range(12):
                                nc.tensor.matmul(
                                    ps[:], wqbt[:, k, 128 * m : 128 * (m + 1)],
                                    qlt[:, k], start=(k == 0), stop=(k == 11),
                                )
                            dst = qTp[:, m, r] if m < 4 else qrw[:, m - 4, r]
                            nc.any.tensor_copy(
                                dst[:, c * HB : (c + 1) * HB], ps[:]
                            )
                        # rope q: sel-matmul + CS-mul; the [rot;swap] pair-
                        # combine folds into the score matmul via krotT2
                        for h in range(HPC):
                            sp = psS.tile([128, HB], f32, tag="big",
                                          name="selps")
                            nc.tensor.matmul(
                                sp[:], selt[:, h % 2],
                                qrw[:, h // 2, r, c * HB : (c + 1) * HB],
                                start=True, stop=True)
                            nc.vector.tensor_mul(
                                qrot[h][:, NB * r + c * HB : NB * r + (c + 1) * HB],
                                sp[:],
                                cst[:, NB * r + c * HB : NB * r + (c + 1) * HB],
                            )
                        qb = 2 * r + c
                        attention_pair((0, 1), qb)
                        attention_pair((2, 3), qb)
                        # o_proj partial for this query block
                        for half2 in range(2):
                            tm = 4 * r + 2 * c + half2
                            s = 128 * half2
                            for n in range(4):
                                ps = psS.tile([128, 512], f32, tag="big",
                                              name="ops")
                                for k in range(4):
                                    nc.tensor.matmul(
                                        ps[:], attnT[:, k, qb, s : s + 128],
                                        wot[:, k, 512 * n : 512 * (n + 1)],
                                        start=(k == 0), stop=(k == 3),
                                    )
                                ot = p2eo.tile([128, 512], f32, tag="oT")
                                nc.any.tensor_copy(ot[:], ps[:])
                                nc.sync.dma_start(
                                    out[128 * tm : 128 * (tm + 1),
                                        512 * n : 512 * (n + 1)], ot[:],
                                )

    nc.finalize()
    return nc


_NC = None


def _get_nc():
    global _NC
    if _NC is None:
        _NC = _build()
    return _NC


def _prep_inputs(x, attention_mask, positions, wqa, qa_scale, wqb, wkva,
                 kva_scale, wkvb, wo):
    x = np.asarray(x, np.float32)
    positions = np.asarray(positions)
    wqa = np.asarray(wqa, np.float32)
    wqb = np.asarray(wqb, np.float32) * np.asarray(qa_scale, np.float32)[:, None]
    wkva = np.asarray(wkva, np.float32)
    wkvb = np.asarray(wkvb, np.float32) * np.asarray(kva_scale, np.float32)[:, None]
    wo = np.asarray(wo, np.float32)

    # wkva augmented with swapped-rot columns
    kr = wkva[:, KVLR:]
    wkva_aug = np.concatenate(
        [wkva[:, :KVLR], kr, kr[:, DR // 2 :], kr[:, : DR // 2]], axis=1
    ).astype(_BF)

    # triu mask for the 128-col diagonal window: mask[r, c] = c >= r
    rr = np.arange(128)[:, None]
    cc = np.arange(128)[None, :]
    maskp = (cc >= rr).astype(_BF)

    eye2 = np.concatenate([np.eye(64), np.eye(64)], axis=0).astype(_BF)

    # sel[v]: out rows [0:64]=src rows [64v:64v+64]; [64:128]=32-swapped copy
    sel = np.zeros((2, 128, 128), np.float32)
    for v in range(2):
        for i in range(64):
            sel[v, 64 * v + i, i] = 1.0
            sel[v, 64 * v + ((i + 32) % 64), 64 + i] = 1.0
    sel = sel.astype(_BF)

    # per-batch cos/sin stack [c; c; -s; s]
    inv_freq = 1.0 / (THETA ** (np.arange(0, DR, 2, dtype=np.float32) / DR))
    cs_b = []
    for b in range(B):
        ang = positions[b].astype(np.float32)[None, :] * inv_freq[:, None]
        c, s = np.cos(ang), np.sin(ang)
        cs_b.append(np.concatenate([c, c, -s, s], axis=0).astype(_BF))

    wqa_bf = wqa.astype(_BF)
    in_maps = []
    for core in range(8):
        b, j = core // 4, core % 4
        hs = [4 * (core % 4) + i for i in range(HPC)]
        # wqb per head-group: [pass x4 | rot per head x4]
        cols = [wqb[:, h * DQK : h * DQK + DN] for h in hs]
        for h in hs:
            cols.append(wqb[:, h * DQK + DN : (h + 1) * DQK])
        wqb_hg = np.concatenate(cols, axis=1).astype(_BF)
        wkvk_hg = np.concatenate(
            [wkvb[:, h * (DN + DV) : h * (DN + DV) + DN] for h in hs], axis=1
        ).astype(_BF)
        wkvv_hg = np.concatenate(
            [wkvb[:, h * (DN + DV) + DN : (h + 1) * (DN + DV)] for h in hs], axis=1
        ).astype(_BF)
        wo_hg = wo[hs[0] * DV : (hs[-1] + 1) * DV, :].astype(_BF)
        xTb = np.ascontiguousarray(
            x[b, NB * j : NB * (j + 1), :].T).astype(_BF)
        in_maps.append({
            "xT": xTb,
            "wqa": wqa_bf,
            "wkva": wkva_aug,
            "wqb": wqb_hg,
            "wkvk": wkvk_hg,
            "wkvv": wkvv_hg,
            "wo": wo_hg,
            "cs": cs_b[b],
            "cso": np.ascontiguousarray(cs_b[b][:, NB * j : NB * (j + 1)]),
            "maskp": maskp,
            "eye2": eye2,
            "sel": sel,
        })
    return in_maps


def _run(inputs, trace=False, trace_kwargs=None):
    from concourse.bass_utils import run_bass_kernel_spmd

    nc = _get_nc()
    in_maps = _prep_inputs(**inputs)
    res = run_bass_kernel_spmd(
        nc, in_maps, list(range(8)), trace=trace,
        trace_kwargs=trace_kwargs or {},
    )
    outs = np.zeros((B, T, HID), np.float32)
    for core in range(8):
        outs[core // 4] += res.results[core]["out"]
    return outs, res


def kernel(**inputs) -> np.ndarray:
    out, _ = _run(inputs)
    return out
